# revision 41
# baseline (speedup 1.0000x reference)
"""Trainium2 Bass kernel for a binarized ResNet BasicBlock (stride-2).

Reference computation (per image):
    residual = BN2(conv1x1(avgpool2x2(x), w_ds))          # full precision
    body     = BN1(conv3x3_s2_p1(sign(x), sign(w_body)))  # binarized
    out      = body + residual

Shapes: x [16, 32, 224, 224] f32 -> out [16, 64, 112, 112] f32.
Sharding: data-parallel over batch, 2 images per core on 8 cores.

v4 layout (all-fp8, all-DoubleRow). Per chunk PAIR (two vertically adjacent
8-output-row chunks of one image; the even chunk's rows live on SBUF
partitions 0:64, the odd chunk's on 64:128, feeding the two PE row-group
strips concurrently):
  * Host pre-casts the input to fp8e4 (sign bit preserved; the residual
    path tolerates the quantization, |err| ~1e-2 vs tolerance ~2) and
    splits each row into [even columns (112) | odd columns (112)]. One fp8
    DMA per pair loads V [128, 9, 224].
  * S holds sign(x) as +-1 fp8 in the same split-column layout, slot
    stride 240: [pad(2) | even 2:114 | pad 114:116 | odd 116:228]. Two DVE
    tensor_scalar ops per pair compute (v & 0x8080) | 0x3838 on uint16
    views (keeps the DVE 2x packed mode). Pad bytes are zeroed once per
    physical buffer; the kx=0 tap at X=0 reads byte 115.
  * Body matmuls are fp8 DoubleRow: rhs is a custom 4D AP
    [K=64, Ko=2, rows=4, cols=112] where Ko and rows both stride one slot,
    so output row y reads slots (y, y+1): par0 cells see (row 2Y-2, 2Y) ->
    weights (0, w_ky1); par1 cells see (2Y-1, 2Y+1) -> (w_ky0, w_ky2). One
    DR matmul per (kx, 4-row group) covers all three ky taps.
  * Residual matmuls are DoubleRow as well: Ko pairs (even col X, odd col
    X) at step 112 with weights (wr, wr) compute the 2x2-pool 1x1-conv
    column sum; rows stride one V slot. One DR matmul per 4-row group,
    weights pre-scaled by inv2/(4*inv1).
  * DoubleRow requires output column group 0, so BOTH halves write PSUM
    partitions 0:64 (row-tiled, tile_position (0,0)/(64,0)) into one
    shared PSUM tile [64, 4, 512] per pair (t-slices 2q+t); 2 such tiles
    double-buffer into the 8 banks.
  * One ScalarE activation per pair (Identity, scale/bias vectors) applies
    both BNs evacuating PSUM->SBUF f32 [64, 16, 112], and one DMA per pair
    (SP / Activation queues alternating) stores 16 contiguous output rows.
"""

import numpy as np
import ml_dtypes

EPS = 1e-5

# Full-problem constants (hardcoded; the harness provides only kernel.py).
B, CIN, COUT, H, W = 16, 32, 64, 224, 224
N_CORES = 8
B_CORE = B // N_CORES  # 2 images per core

CHUNK_ROWS = 8
SPAD = 240   # padded S row-slot stride (fp8 bytes), %16 == 0 for DoubleRow
SEVEN = 2    # S even-column block byte offset
SODD = 116   # S odd-column block byte offset (kx=0 pad byte at 115)


def build_nc(b_core=B_CORE, cin=CIN, cout=COUT, h=H, w=W,
             chunk_rows=CHUNK_ROWS, loop_reps=1, ablate=None):
    """Build the Bass program for one core processing b_core images.

    loop_reps > 1 wraps the whole computation in a hardware loop (identical
    results each iteration) — used only for wall-clock timing amplification.
    """
    from contextlib import nullcontext
    import concourse.bass as bass
    import concourse.bacc as bacc
    import concourse.mybir as mybir
    import concourse.tile as tile

    ho, wo = h // 2, w // 2
    assert ho % chunk_rows == 0
    n_chunks = ho // chunk_rows
    assert chunk_rows % 4 == 0
    T = chunk_rows // 4  # 4 output rows per matmul tile
    nslots = chunk_rows + 1  # one extra leading row slot per chunk

    f32 = mybir.dt.float32
    bf16 = mybir.dt.bfloat16
    fp8 = mybir.dt.float8e4
    u16 = mybir.dt.uint16
    DR = mybir.MatmulPerfMode.DoubleRow

    nc = bacc.Bacc("TRN2", target_bir_lowering=False, debug=False)

    # Input is pre-arranged on the host as one payload per two chunk PAIRS
    # (one DMA feeds two pairs): zz[dp, p, pp, slot, u] fp8, partitions
    # 0:64 = even chunk's rows ((par, ci) major, slot = leading-row + 8
    # rows, u = even|odd column split), 64:128 = odd chunk's.
    n_pairs = (b_core * n_chunks + 1) // 2
    n_dp = (n_pairs + 1) // 2
    zz = nc.dram_tensor("zz", [n_dp, 128, 2, nslots, w], fp8,
                        kind="ExternalInput")
    # DoubleRow body weights [p, kx, ko, co]; partitions 64:128 duplicate
    # 0:64 so each PE row group loads from its own partition half.
    w_dr = nc.dram_tensor("w_dr", [128, 3, 2, cout], fp8, kind="ExternalInput")
    w_res = nc.dram_tensor("w_res", [128, 2, cout], fp8, kind="ExternalInput")
    bn_sb = nc.dram_tensor("bn_sb", [cout, 2], f32, kind="ExternalInput")
    # Output is stored bf16 (host upcasts to f32 after gathering): halves
    # the HBM write traffic; rounding error ~0.4% of the output scale.
    out = nc.dram_tensor("out", [b_core, cout, ho, wo], bf16,
                         kind="ExternalOutput")

    def window_ap(base, ko_step, nrows, row_step):
        # [K=64, Ko=2, rows, cols] built from a [K, 1 or 2, cols] slice.
        return bass.AP(base.tensor, base.offset,
                       [list(base.ap[0]), [ko_step, 2], [row_step, nrows],
                        list(base.ap[-1])])

    with tile.TileContext(nc) as tc:
        with tc.tile_pool(name="consts", bufs=1) as cpool:
            wdr = cpool.tile([128, 3, 2, cout], fp8)
            nc.sync.dma_start(out=wdr[:, :, :, :], in_=w_dr.ap()[:, :, :, :])
            wrd = cpool.tile([128, 2, cout], fp8)
            nc.scalar.dma_start(out=wrd[:, :, :], in_=w_res.ap()[:, :, :])
            sb_ = cpool.tile([cout, 2], f32)
            nc.sync.dma_start(out=sb_[:, :], in_=bn_sb.ap()[:, :])
            sc, bi = sb_[:, 0:1], sb_[:, 1:2]

            G = b_core * n_chunks
            n_pairs_r = (G + 1) // 2
            with (
                tc.tile_pool(name="vpool", bufs=n_dp) as vpool,
                tc.tile_pool(name="spool", bufs=1) as spool,
                tc.tile_pool(name="opool", bufs=6) as opool,
                tc.tile_pool(name="pspool", bufs=4, space="PSUM") as pspool,
            ):
                # One dedicated S region per pair inside a single tile (not
                # pool-cycled) so the zero-pad bytes initialize in a handful
                # of DVE ops (startup latency) and all sign ops can be
                # issued upfront each iteration. For pairs whose even chunk
                # is c == 0, q0's slot 0 is the conv's zero padding row (the
                # sign op skips it; zeroed here once).
                s_all = spool.tile([128, n_pairs_r, nslots, SPAD], fp8,
                                   name="s_all")
                nc.vector.memset(s_all[:, :, :, 0:SEVEN], 0.0)
                nc.vector.memset(s_all[:, :, :, SEVEN + w // 2 : SODD], 0.0)
                s_bufs = [s_all[:, si] for si in range(n_pairs_r)]
                for si in range(n_pairs_r):
                    if (2 * si) % n_chunks == 0:
                        nc.vector.memset(s_all[0:64, si, 0:1, :], 0.0)
                v_bufs = []
                if ablate == "no_in":
                    # Pre-initialized V stand-ins so the compute path can be
                    # timed without the input DMAs.
                    for vi in range(n_dp):
                        vb = spool.tile([128, 2, nslots, w], fp8,
                                        name=f"vbuf{vi}")
                        nc.vector.memset(vb[:, :, :, :], 0.25)
                        v_bufs.append(vb)
                    nc.vector.memset(
                        s_all[:, :, :, SEVEN : SEVEN + w // 2], 1.0)
                    nc.vector.memset(s_all[:, :, :, SODD : SODD + w // 2], 1.0)

                reps_ctx = (
                    tc.For_i(0, loop_reps, 1) if loop_reps > 1 else nullcontext()
                )
                with reps_ctx:
                  # Phase 1: all input DMAs + sign ops issued upfront (each
                  # pair has its own S buffer), so the DVE finishes the
                  # latency-critical signs early and can absorb evacuation
                  # work for the later pairs.
                  v2s = []
                  for dp in range(n_dp):
                    v2 = (v_bufs[dp] if ablate == "no_in"
                          else vpool.tile([128, 2, nslots, w], fp8, name="v2"))
                    v2s.append(v2)
                    if ablate != "no_in":
                        if dp == 0:
                            # Split the first payload so the first sign op
                            # (and the PE pipeline) starts half a transfer
                            # earlier.
                            for hh in range(2):
                                nc.gpsimd.dma_start(
                                    out=v2[:, hh, :, :],
                                    in_=zz.ap()[dp, :, hh, :, :])
                        else:
                            nc.gpsimd.dma_start(out=v2[:, :, :, :],
                                                in_=zz.ap()[dp, :, :, :, :])
                  if ablate != "no_in":
                    for pair in range(n_pairs):
                        c0_pair = (2 * pair) % n_chunks == 0
                        h = pair % 2
                        s, v2 = s_bufs[pair], v2s[pair // 2]
                        # sign bits: s = (v & 0x8080) | 0x3838 (+-1 fp8), on
                        # u16 views; one op per column-parity block. For a
                        # c == 0 pair, q0's slot 0 (padding) must stay zero.
                        for plo, phi, jlo in (
                            [(0, 64, 1), (64, 128, 0)] if c0_pair
                            else [(0, 128, 0)]
                        ):
                            for so, vo in ((SEVEN, 0), (SODD, w // 2)):
                                nc.vector.tensor_scalar(
                                    s.bitcast(u16)[plo:phi, jlo:,
                                                   so // 2 : (so + w // 2) // 2],
                                    v2.bitcast(u16)[plo:phi, h, jlo:,
                                                    vo // 2 : (vo + w // 2) // 2],
                                    0x8080,
                                    0x3838,
                                    mybir.AluOpType.bitwise_and,
                                    mybir.AluOpType.bitwise_or,
                                )
                  # Phase 2: matmuls + evacuation + output DMAs per pair.
                  for pair in range(n_pairs):
                    halves = [q for q in range(2) if 2 * pair + q < G]
                    h = pair % 2
                    s, v2 = s_bufs[pair], v2s[pair // 2]
                    ps = {q: pspool.tile([64, T, 512], f32, name=f"ps{q}",
                                         tag="ps")
                          for q in halves}
                    if ablate not in ("io_only",):
                        for kx, so in ((0, SODD - 1), (1, SEVEN), (2, SODD)):
                            for t in range(T):
                                for q in halves:
                                    p0 = 64 * q
                                    base = s[p0 : p0 + 64, 4 * t : 4 * t + 2,
                                             so : so + wo]
                                    nc.tensor.matmul(
                                        ps[q][0:64, t, 0 : 4 * wo],
                                        wdr[p0 : p0 + 64, kx, :, :],
                                        window_ap(base, SPAD, 4, SPAD),
                                        start=(kx == 0), stop=False,
                                        perf_mode=DR,
                                        tile_position=(p0, 0),
                                    )
                        for t in range(T):
                            j0 = 1 + 4 * t
                            for q in halves:
                                p0 = 64 * q
                                base = v2[p0 : p0 + 64, h, j0 : j0 + 1, 0:wo]
                                nc.tensor.matmul(
                                    ps[q][0:64, t, 0 : 4 * wo],
                                    wrd[p0 : p0 + 64, :, :],
                                    window_ap(base, wo, 4, w),
                                    start=False, stop=True,
                                    perf_mode=DR,
                                    tile_position=(p0, 0),
                                )
                        if ablate != "mm_only":
                            # BN + evacuate: out = psum*inv1 + (shift1+shift2).
                            # The odd chunk's evacuation writes SBUF partitions
                            # 64:128 so the two output DMAs hit disjoint SDMA
                            # engine groups (engine assignment is by source
                            # partition).
                            if h == 0:
                                # bf16 staging: the output DMA upcasts to f32
                                # on the fly, halving the SBUF-side read
                                # bytes per SDMA engine.
                                o2 = opool.tile([128, 2, chunk_rows, wo],
                                                bf16, name="o2")
                            for q in halves:
                                p0 = 64 * q
                                oview = o2[p0 : p0 + 64, h].rearrange(
                                    "p (t j) x -> p t (j x)", t=T)
                                if pair < 9:
                                    nc.scalar.activation(
                                        oview,
                                        ps[q][:, :, 0 : 4 * wo],
                                        mybir.ActivationFunctionType.Identity,
                                        bias=bi,
                                        scale=sc,
                                    )
                                else:
                                    # Late pairs: DVE has finished the signs
                                    # by now; share the evacuation load.
                                    nc.vector.tensor_scalar(
                                        oview,
                                        ps[q][:, :, 0 : 4 * wo],
                                        sc,
                                        bi,
                                        mybir.AluOpType.mult,
                                        mybir.AluOpType.add,
                                    )
                            last_grp = pair >= 2 * ((n_pairs - 1) // 2)
                            if ablate != "no_out" and (h == 1
                                                       or pair == n_pairs - 1
                                                       or last_grp):
                                # One DMA per queue covers this group's two
                                # pairs (4 chunks): a custom DRAM AP supplies
                                # the chunk stride (group may span images).
                                # The final group instead stores per pair so
                                # the drain tail is a single small transfer.
                                for q in halves:
                                    if last_grp:
                                        gs = [2 * pair + q]
                                    else:
                                        gs = [2 * pp_ + q for pp_ in
                                              (pair - h, pair)
                                              if 2 * pp_ + q < G]
                                    b1, c1 = divmod(gs[0], n_chunks)
                                    y01 = c1 * chunk_rows
                                    base = out.ap()[b1, :,
                                                    y01 : y01 + chunk_rows, :]
                                    if len(gs) == 2:
                                        b2, c2 = divmod(gs[1], n_chunks)
                                        spp = (((b2 - b1) * cout * ho)
                                               + (c2 - c1) * chunk_rows) * wo
                                        dst = bass.AP(
                                            base.tensor, base.offset,
                                            [list(base.ap[0]), [spp, 2],
                                             list(base.ap[1]),
                                             list(base.ap[2])])
                                        src = o2[64 * q : 64 * q + 64, :, :, :]
                                    else:
                                        dst = base
                                        src = o2[64 * q : 64 * q + 64, h, :, :]
                                    out_eng = nc.sync if q == 0 else nc.scalar
                                    out_eng.dma_start(out=dst, in_=src)
    nc.compile()
    return nc


def prep_weights(w_body, w_ds, bn1_gamma, bn1_beta, bn1_mean, bn1_var,
                 bn2_gamma, bn2_beta, bn2_mean, bn2_var):
    """Host-side parameter folding (all small tensors)."""
    fp8 = ml_dtypes.float8_e4m3
    cout, cin = w_body.shape[0], w_body.shape[1]
    inv1 = (bn1_gamma / np.sqrt(bn1_var + EPS)).astype(np.float32)
    inv2 = (bn2_gamma / np.sqrt(bn2_var + EPS)).astype(np.float32)
    shift1 = (bn1_beta - bn1_mean * inv1).astype(np.float32)
    shift2 = (bn2_beta - bn2_mean * inv2).astype(np.float32)

    wb_sign = np.where(w_body >= 0, 1.0, -1.0).astype(np.float32)  # [co,ci,ky,kx]

    # DoubleRow body weights [p, kx, ko, co]: par0 rows hold (0, w_ky1)
    # (slot j is row 2Y-2, unwanted), par1 rows hold (w_ky0, w_ky2).
    wdr = np.zeros((128, 3, 2, cout), np.float32)
    for kx in range(3):
        wdr[0:cin, kx, 1] = wb_sign[:, :, 1, kx].T          # par0, ko=1: ky1
        wdr[cin : 2 * cin, kx, 0] = wb_sign[:, :, 0, kx].T  # par1, ko=0: ky0
        wdr[cin : 2 * cin, kx, 1] = wb_sign[:, :, 2, kx].T  # par1, ko=1: ky2
    wdr[64:128] = wdr[0:64]

    # Residual weights with BN2 folded and divided by BN1 scale (the final
    # activation multiplies everything by inv1); identical on both Ko lanes
    # (even + odd column of the 2x2 pool).
    wres = w_ds[:, :, 0, 0] * (inv2 / (4.0 * inv1))[:, None]  # [co, ci]
    w_res = np.tile(wres.T[:, None, :], (4, 2, 1)).reshape(128, 2, cout)

    return dict(
        w_dr=wdr.astype(fp8),
        w_res=w_res.astype(fp8),
        bn_sb=np.stack([inv1, shift1 + shift2], axis=1),
    )


def make_zz(x, cin=CIN, h=H, w=W, chunk_rows=CHUNK_ROWS):
    """Host layout prep: per-chunk-pair fp8 DMA payloads.

    x: [b, ci, r, u] f32. Returns zz[pair, p, slot, u] fp8 where partition
    p = 64*(chunk parity) + par*ci-major, slot j holds input row
    2*(chunk_rows*c - 1 + j) + par split as [even cols | odd cols]; the
    leading slot of chunk 0 is zero padding.
    """
    b_core = x.shape[0]
    hh = h // 2
    n_chunks = hh // chunk_rows
    ns = chunk_rows + 1
    xv = x.reshape(b_core, cin, hh, 2, w // 2, 2).transpose(0, 3, 1, 2, 5, 4)
    # xv: [b, par, ci, r2, colpar, u'] -> rows split into even|odd columns
    xv = xv.reshape(b_core, 2 * cin, hh, w).astype(ml_dtypes.float8_e4m3)
    G = b_core * n_chunks
    n_pairs = (G + 1) // 2
    zz = np.zeros(((n_pairs + 1) // 2, 128, 2, ns, w), ml_dtypes.float8_e4m3)
    for g in range(G):
        b, c = divmod(g, n_chunks)
        q, y0 = g % 2, c * chunk_rows
        jlo = 1 if c == 0 else 0
        pair = g // 2
        zz[pair // 2, 64 * q : 64 * q + 64, pair % 2, jlo:ns] = xv[
            b, :, y0 - 1 + jlo : y0 + chunk_rows, :]
    return zz


def kernel(x, w_body, bn1_gamma, bn1_beta, bn1_mean, bn1_var,
           w_ds, bn2_gamma, bn2_beta, bn2_mean, bn2_var):
    from concourse.bass_utils import run_bass_kernel_spmd

    x = np.asarray(x, dtype=np.float32)
    params = prep_weights(
        np.asarray(w_body, np.float32), np.asarray(w_ds, np.float32),
        np.asarray(bn1_gamma, np.float32), np.asarray(bn1_beta, np.float32),
        np.asarray(bn1_mean, np.float32), np.asarray(bn1_var, np.float32),
        np.asarray(bn2_gamma, np.float32), np.asarray(bn2_beta, np.float32),
        np.asarray(bn2_mean, np.float32), np.asarray(bn2_var, np.float32),
    )

    nc = build_nc()
    in_maps = [
        {"zz": make_zz(x[k * B_CORE : (k + 1) * B_CORE]), **params}
        for k in range(N_CORES)
    ]
    res = run_bass_kernel_spmd(nc, in_maps, core_ids=list(range(N_CORES)))
    return np.concatenate(
        [np.asarray(r["out"], dtype=np.float32) for r in res.results], axis=0)


# revision 42
# speedup vs baseline: 1.1436x; 1.1436x over previous
"""Trainium2 Bass kernel for a binarized ResNet BasicBlock (stride-2).

Reference computation (per image):
    residual = BN2(conv1x1(avgpool2x2(x), w_ds))          # full precision
    body     = BN1(conv3x3_s2_p1(sign(x), sign(w_body)))  # binarized
    out      = body + residual

Shapes: x [16, 32, 224, 224] f32 -> out [16, 64, 112, 112] f32.
Sharding: data-parallel over batch, 2 images per core on 8 cores.

Layout (all-fp8, all-DoubleRow). Per chunk PAIR (two vertically adjacent
8-output-row chunks of one image; the even chunk's rows live on SBUF
partitions 0:64, the odd chunk's on 64:128, feeding the two PE row-group
strips concurrently):
  * Host pre-casts the input to fp8e4 (sign bit preserved; the residual
    path tolerates the quantization) and splits each row into
    [even columns (112) | odd columns (112)]. One fp8 DMA per TWO pairs
    (gpsimd/SWDGE queue) loads V [128, 2, 9, 224]; the first payload is
    split in half so the pipeline fills earlier.
  * S holds sign(x) as +-1 fp8 in the same split-column layout, slot
    stride 240: [pad(2) | even 2:114 | pad 114:116 | odd 116:228], one
    dedicated region per pair inside a single SBUF tile (pads zeroed in a
    handful of startup memsets). All sign ops are issued upfront each
    iteration: DVE tensor_scalar computes (v & 0x8080) | 0x3838 on uint16
    views (keeps the DVE 2x packed mode). The kx=0 tap at X=0 reads the
    zero pad byte 115; a c==0 chunk's padding row stays zero because its
    sign op skips slot 0.
  * Body matmuls are fp8 DoubleRow: rhs is a custom 4D AP
    [K=64, Ko=2, rows=4, cols=112] where Ko and rows both stride one slot,
    so output row y reads slots (y, y+1): par0 cells see (row 2Y-2, 2Y) ->
    weights (0, w_ky1); par1 cells see (2Y-1, 2Y+1) -> (w_ky0, w_ky2). One
    DR matmul per (kx, 4-row group) covers all three ky taps.
  * Residual matmuls are DoubleRow as well: Ko pairs (even col X, odd col
    X) at step 112 with weights (wr, wr) compute the 2x2-pool 1x1-conv
    column sum; rows stride one V slot. One DR matmul per 4-row group,
    weights pre-scaled by inv2/(4*inv1).
  * DoubleRow requires output column group 0, so BOTH halves write PSUM
    partitions 0:64 (row-tiled, tile_position (0,0)/(64,0)) into per-chunk
    PSUM tiles [64, 2, 512]; 4 such tiles double-buffer into the 8 banks.
  * BN evacuation (out = psum*inv1 + shift1+shift2) is per chunk, writing
    bf16: ScalarE activation for early pairs, DVE tensor_scalar with
    vector scale/bias for late pairs (after the upfront signs drain). The
    odd chunk's evacuation writes SBUF partitions 64:128 so the two output
    queues (SP / Activation) hit disjoint SDMA engine groups (engine
    assignment is by source partition).
  * Output DMAs are batched two pairs per queue via a custom strided DRAM
    AP (the group may span images); the final group stores per pair for a
    short drain. The DRAM output is bf16; the host upcasts to f32 after
    gathering, halving HBM write traffic (rounding ~0.4% of output scale).
"""

import numpy as np
import ml_dtypes

EPS = 1e-5

# Full-problem constants (hardcoded; the harness provides only kernel.py).
B, CIN, COUT, H, W = 16, 32, 64, 224, 224
N_CORES = 8
B_CORE = B // N_CORES  # 2 images per core

CHUNK_ROWS = 8
SPAD = 240   # padded S row-slot stride (fp8 bytes), %16 == 0 for DoubleRow
SEVEN = 2    # S even-column block byte offset
SODD = 116   # S odd-column block byte offset (kx=0 pad byte at 115)


def build_nc(b_core=B_CORE, cin=CIN, cout=COUT, h=H, w=W,
             chunk_rows=CHUNK_ROWS, loop_reps=1, ablate=None):
    """Build the Bass program for one core processing b_core images.

    loop_reps > 1 wraps the whole computation in a hardware loop (identical
    results each iteration) — used only for wall-clock timing amplification.
    """
    from contextlib import nullcontext
    import concourse.bass as bass
    import concourse.bacc as bacc
    import concourse.mybir as mybir
    import concourse.tile as tile

    ho, wo = h // 2, w // 2
    assert ho % chunk_rows == 0
    n_chunks = ho // chunk_rows
    assert chunk_rows % 4 == 0
    T = chunk_rows // 4  # 4 output rows per matmul tile
    nslots = chunk_rows + 1  # one extra leading row slot per chunk

    f32 = mybir.dt.float32
    bf16 = mybir.dt.bfloat16
    fp8 = mybir.dt.float8e4
    u16 = mybir.dt.uint16
    DR = mybir.MatmulPerfMode.DoubleRow

    nc = bacc.Bacc("TRN2", target_bir_lowering=False, debug=False)

    # Input is pre-arranged on the host as one payload per two chunk PAIRS
    # (one DMA feeds two pairs): zz[dp, p, pp, slot, u] fp8, partitions
    # 0:64 = even chunk's rows ((par, ci) major, slot = leading-row + 8
    # rows, u = even|odd column split), 64:128 = odd chunk's.
    n_pairs = (b_core * n_chunks + 1) // 2
    n_dp = (n_pairs + 1) // 2
    zz = nc.dram_tensor("zz", [n_dp, 128, 2, nslots, w], fp8,
                        kind="ExternalInput")
    # DoubleRow body weights [p, kx, ko, co]; partitions 64:128 duplicate
    # 0:64 so each PE row group loads from its own partition half.
    w_dr = nc.dram_tensor("w_dr", [128, 3, 2, cout], fp8, kind="ExternalInput")
    w_res = nc.dram_tensor("w_res", [128, 2, cout], fp8, kind="ExternalInput")
    bn_sb = nc.dram_tensor("bn_sb", [cout, 2], f32, kind="ExternalInput")
    # Output is stored bf16 (host upcasts to f32 after gathering): halves
    # the HBM write traffic; rounding error ~0.4% of the output scale.
    out = nc.dram_tensor("out", [b_core, cout, ho, wo], bf16,
                         kind="ExternalOutput")

    def window_ap(base, ko_step, nrows, row_step):
        # [K=64, Ko=2, rows, cols] built from a [K, 1 or 2, cols] slice.
        return bass.AP(base.tensor, base.offset,
                       [list(base.ap[0]), [ko_step, 2], [row_step, nrows],
                        list(base.ap[-1])])

    with tile.TileContext(nc) as tc:
        with tc.tile_pool(name="consts", bufs=1) as cpool:
            wdr = cpool.tile([128, 3, 2, cout], fp8)
            nc.sync.dma_start(out=wdr[:, :, :, :], in_=w_dr.ap()[:, :, :, :])
            wrd = cpool.tile([128, 2, cout], fp8)
            nc.scalar.dma_start(out=wrd[:, :, :], in_=w_res.ap()[:, :, :])
            sb_ = cpool.tile([cout, 2], f32)
            nc.sync.dma_start(out=sb_[:, :], in_=bn_sb.ap()[:, :])
            sc, bi = sb_[:, 0:1], sb_[:, 1:2]

            G = b_core * n_chunks
            n_pairs_r = (G + 1) // 2
            with (
                tc.tile_pool(name="vpool", bufs=n_dp) as vpool,
                tc.tile_pool(name="spool", bufs=1) as spool,
                tc.tile_pool(name="opool", bufs=6) as opool,
                tc.tile_pool(name="pspool", bufs=4, space="PSUM") as pspool,
            ):
                # One dedicated S region per pair inside a single tile (not
                # pool-cycled) so the zero-pad bytes initialize in a handful
                # of DVE ops (startup latency) and all sign ops can be
                # issued upfront each iteration. For pairs whose even chunk
                # is c == 0, q0's slot 0 is the conv's zero padding row (the
                # sign op skips it; zeroed here once).
                s_all = spool.tile([128, n_pairs_r, nslots, SPAD], fp8,
                                   name="s_all")
                nc.vector.memset(s_all[:, :, :, 0:SEVEN], 0.0)
                nc.vector.memset(s_all[:, :, :, SEVEN + w // 2 : SODD], 0.0)
                s_bufs = [s_all[:, si] for si in range(n_pairs_r)]
                for si in range(n_pairs_r):
                    if (2 * si) % n_chunks == 0:
                        nc.vector.memset(s_all[0:64, si, 0:1, :], 0.0)
                v_bufs = []
                if ablate == "no_in":
                    # Pre-initialized V stand-ins so the compute path can be
                    # timed without the input DMAs.
                    for vi in range(n_dp):
                        vb = spool.tile([128, 2, nslots, w], fp8,
                                        name=f"vbuf{vi}")
                        nc.vector.memset(vb[:, :, :, :], 0.25)
                        v_bufs.append(vb)
                    nc.vector.memset(
                        s_all[:, :, :, SEVEN : SEVEN + w // 2], 1.0)
                    nc.vector.memset(s_all[:, :, :, SODD : SODD + w // 2], 1.0)

                reps_ctx = (
                    tc.For_i(0, loop_reps, 1) if loop_reps > 1 else nullcontext()
                )
                with reps_ctx:
                  # Phase 1: all input DMAs + sign ops issued upfront (each
                  # pair has its own S buffer), so the DVE finishes the
                  # latency-critical signs early and can absorb evacuation
                  # work for the later pairs.
                  v2s = []
                  for dp in range(n_dp):
                    v2 = (v_bufs[dp] if ablate == "no_in"
                          else vpool.tile([128, 2, nslots, w], fp8, name="v2"))
                    v2s.append(v2)
                    if ablate != "no_in":
                        if dp == 0:
                            # Split the first payload so the first sign op
                            # (and the PE pipeline) starts half a transfer
                            # earlier.
                            for hh in range(2):
                                nc.gpsimd.dma_start(
                                    out=v2[:, hh, :, :],
                                    in_=zz.ap()[dp, :, hh, :, :])
                        else:
                            nc.gpsimd.dma_start(out=v2[:, :, :, :],
                                                in_=zz.ap()[dp, :, :, :, :])
                  if ablate != "no_in":
                    for pair in range(n_pairs):
                        c0_pair = (2 * pair) % n_chunks == 0
                        h = pair % 2
                        s, v2 = s_bufs[pair], v2s[pair // 2]
                        # sign bits: s = (v & 0x8080) | 0x3838 (+-1 fp8), on
                        # u16 views; one op per column-parity block. For a
                        # c == 0 pair, q0's slot 0 (padding) must stay zero.
                        for plo, phi, jlo in (
                            [(0, 64, 1), (64, 128, 0)] if c0_pair
                            else [(0, 128, 0)]
                        ):
                            for so, vo in ((SEVEN, 0), (SODD, w // 2)):
                                nc.vector.tensor_scalar(
                                    s.bitcast(u16)[plo:phi, jlo:,
                                                   so // 2 : (so + w // 2) // 2],
                                    v2.bitcast(u16)[plo:phi, h, jlo:,
                                                    vo // 2 : (vo + w // 2) // 2],
                                    0x8080,
                                    0x3838,
                                    mybir.AluOpType.bitwise_and,
                                    mybir.AluOpType.bitwise_or,
                                )
                  # Phase 2: matmuls + evacuation + output DMAs per pair.
                  for pair in range(n_pairs):
                    halves = [q for q in range(2) if 2 * pair + q < G]
                    h = pair % 2
                    s, v2 = s_bufs[pair], v2s[pair // 2]
                    ps = {q: pspool.tile([64, T, 512], f32, name=f"ps{q}",
                                         tag="ps")
                          for q in halves}
                    if ablate not in ("io_only",):
                        for kx, so in ((0, SODD - 1), (1, SEVEN), (2, SODD)):
                            for t in range(T):
                                for q in halves:
                                    p0 = 64 * q
                                    base = s[p0 : p0 + 64, 4 * t : 4 * t + 2,
                                             so : so + wo]
                                    nc.tensor.matmul(
                                        ps[q][0:64, t, 0 : 4 * wo],
                                        wdr[p0 : p0 + 64, kx, :, :],
                                        window_ap(base, SPAD, 4, SPAD),
                                        start=(kx == 0), stop=False,
                                        perf_mode=DR,
                                        tile_position=(p0, 0),
                                    )
                        for t in range(T):
                            j0 = 1 + 4 * t
                            for q in halves:
                                p0 = 64 * q
                                base = v2[p0 : p0 + 64, h, j0 : j0 + 1, 0:wo]
                                nc.tensor.matmul(
                                    ps[q][0:64, t, 0 : 4 * wo],
                                    wrd[p0 : p0 + 64, :, :],
                                    window_ap(base, wo, 4, w),
                                    start=False, stop=True,
                                    perf_mode=DR,
                                    tile_position=(p0, 0),
                                )
                        if ablate != "mm_only":
                            # BN + evacuate: out = psum*inv1 + (shift1+shift2).
                            # The odd chunk's evacuation writes SBUF partitions
                            # 64:128 so the two output DMAs hit disjoint SDMA
                            # engine groups (engine assignment is by source
                            # partition).
                            if h == 0:
                                # bf16 staging: the output DMA upcasts to f32
                                # on the fly, halving the SBUF-side read
                                # bytes per SDMA engine.
                                o2 = opool.tile([128, 2, chunk_rows, wo],
                                                bf16, name="o2")
                            for q in halves:
                                p0 = 64 * q
                                oview = o2[p0 : p0 + 64, h].rearrange(
                                    "p (t j) x -> p t (j x)", t=T)
                                if pair < 9:
                                    nc.scalar.activation(
                                        oview,
                                        ps[q][:, :, 0 : 4 * wo],
                                        mybir.ActivationFunctionType.Identity,
                                        bias=bi,
                                        scale=sc,
                                    )
                                else:
                                    # Late pairs: DVE has finished the signs
                                    # by now; share the evacuation load.
                                    nc.vector.tensor_scalar(
                                        oview,
                                        ps[q][:, :, 0 : 4 * wo],
                                        sc,
                                        bi,
                                        mybir.AluOpType.mult,
                                        mybir.AluOpType.add,
                                    )
                            last_grp = pair >= 2 * ((n_pairs - 1) // 2)
                            if ablate != "no_out" and (h == 1
                                                       or pair == n_pairs - 1
                                                       or last_grp):
                                # One DMA per queue covers this group's two
                                # pairs (4 chunks): a custom DRAM AP supplies
                                # the chunk stride (group may span images).
                                # The final group instead stores per pair so
                                # the drain tail is a single small transfer.
                                for q in halves:
                                    if last_grp:
                                        gs = [2 * pair + q]
                                    else:
                                        gs = [2 * pp_ + q for pp_ in
                                              (pair - h, pair)
                                              if 2 * pp_ + q < G]
                                    b1, c1 = divmod(gs[0], n_chunks)
                                    y01 = c1 * chunk_rows
                                    base = out.ap()[b1, :,
                                                    y01 : y01 + chunk_rows, :]
                                    if len(gs) == 2:
                                        b2, c2 = divmod(gs[1], n_chunks)
                                        spp = (((b2 - b1) * cout * ho)
                                               + (c2 - c1) * chunk_rows) * wo
                                        dst = bass.AP(
                                            base.tensor, base.offset,
                                            [list(base.ap[0]), [spp, 2],
                                             list(base.ap[1]),
                                             list(base.ap[2])])
                                        src = o2[64 * q : 64 * q + 64, :, :, :]
                                    else:
                                        dst = base
                                        src = o2[64 * q : 64 * q + 64, h, :, :]
                                    out_eng = nc.sync if q == 0 else nc.scalar
                                    out_eng.dma_start(out=dst, in_=src)
    nc.compile()
    return nc


def prep_weights(w_body, w_ds, bn1_gamma, bn1_beta, bn1_mean, bn1_var,
                 bn2_gamma, bn2_beta, bn2_mean, bn2_var):
    """Host-side parameter folding (all small tensors)."""
    fp8 = ml_dtypes.float8_e4m3
    cout, cin = w_body.shape[0], w_body.shape[1]
    inv1 = (bn1_gamma / np.sqrt(bn1_var + EPS)).astype(np.float32)
    inv2 = (bn2_gamma / np.sqrt(bn2_var + EPS)).astype(np.float32)
    shift1 = (bn1_beta - bn1_mean * inv1).astype(np.float32)
    shift2 = (bn2_beta - bn2_mean * inv2).astype(np.float32)

    wb_sign = np.where(w_body >= 0, 1.0, -1.0).astype(np.float32)  # [co,ci,ky,kx]

    # DoubleRow body weights [p, kx, ko, co]: par0 rows hold (0, w_ky1)
    # (slot j is row 2Y-2, unwanted), par1 rows hold (w_ky0, w_ky2).
    wdr = np.zeros((128, 3, 2, cout), np.float32)
    for kx in range(3):
        wdr[0:cin, kx, 1] = wb_sign[:, :, 1, kx].T          # par0, ko=1: ky1
        wdr[cin : 2 * cin, kx, 0] = wb_sign[:, :, 0, kx].T  # par1, ko=0: ky0
        wdr[cin : 2 * cin, kx, 1] = wb_sign[:, :, 2, kx].T  # par1, ko=1: ky2
    wdr[64:128] = wdr[0:64]

    # Residual weights with BN2 folded and divided by BN1 scale (the final
    # activation multiplies everything by inv1); identical on both Ko lanes
    # (even + odd column of the 2x2 pool).
    wres = w_ds[:, :, 0, 0] * (inv2 / (4.0 * inv1))[:, None]  # [co, ci]
    w_res = np.tile(wres.T[:, None, :], (4, 2, 1)).reshape(128, 2, cout)

    return dict(
        w_dr=wdr.astype(fp8),
        w_res=w_res.astype(fp8),
        bn_sb=np.stack([inv1, shift1 + shift2], axis=1),
    )


def make_zz(x, cin=CIN, h=H, w=W, chunk_rows=CHUNK_ROWS):
    """Host layout prep: per-chunk-pair fp8 DMA payloads.

    x: [b, ci, r, u] f32. Returns zz[pair, p, slot, u] fp8 where partition
    p = 64*(chunk parity) + par*ci-major, slot j holds input row
    2*(chunk_rows*c - 1 + j) + par split as [even cols | odd cols]; the
    leading slot of chunk 0 is zero padding.
    """
    b_core = x.shape[0]
    hh = h // 2
    n_chunks = hh // chunk_rows
    ns = chunk_rows + 1
    xv = x.reshape(b_core, cin, hh, 2, w // 2, 2).transpose(0, 3, 1, 2, 5, 4)
    # xv: [b, par, ci, r2, colpar, u'] -> rows split into even|odd columns
    xv = xv.reshape(b_core, 2 * cin, hh, w).astype(ml_dtypes.float8_e4m3)
    G = b_core * n_chunks
    n_pairs = (G + 1) // 2
    zz = np.zeros(((n_pairs + 1) // 2, 128, 2, ns, w), ml_dtypes.float8_e4m3)
    for g in range(G):
        b, c = divmod(g, n_chunks)
        q, y0 = g % 2, c * chunk_rows
        jlo = 1 if c == 0 else 0
        pair = g // 2
        zz[pair // 2, 64 * q : 64 * q + 64, pair % 2, jlo:ns] = xv[
            b, :, y0 - 1 + jlo : y0 + chunk_rows, :]
    return zz


def kernel(x, w_body, bn1_gamma, bn1_beta, bn1_mean, bn1_var,
           w_ds, bn2_gamma, bn2_beta, bn2_mean, bn2_var):
    from concourse.bass_utils import run_bass_kernel_spmd

    x = np.asarray(x, dtype=np.float32)
    params = prep_weights(
        np.asarray(w_body, np.float32), np.asarray(w_ds, np.float32),
        np.asarray(bn1_gamma, np.float32), np.asarray(bn1_beta, np.float32),
        np.asarray(bn1_mean, np.float32), np.asarray(bn1_var, np.float32),
        np.asarray(bn2_gamma, np.float32), np.asarray(bn2_beta, np.float32),
        np.asarray(bn2_mean, np.float32), np.asarray(bn2_var, np.float32),
    )

    nc = build_nc()
    in_maps = [
        {"zz": make_zz(x[k * B_CORE : (k + 1) * B_CORE]), **params}
        for k in range(N_CORES)
    ]
    res = run_bass_kernel_spmd(nc, in_maps, core_ids=list(range(N_CORES)))
    return np.concatenate(
        [np.asarray(r["out"], dtype=np.float32) for r in res.results], axis=0)
